# revision 1
# baseline (speedup 1.0000x reference)
"""Trainium2 Bass kernel for CrossModalAttentionImproved.

Single-head cross attention + FFN transformer block:
  q = Xq@Wq+bq; k = Xk@Wk+bk; v = Xk@Wv+bv
  attn = softmax(q k^T / sqrt(D)); ctx = attn@v
  out = LN(Xq + ctx@Wo + bo; g1,b1)
  h = gelu(LN(out@W1 + bf1; gf,bf))
  y = LN(out + h@W2 + bf2; g2,b2)

Sharding: data-parallel over batch. B=16 across 8 cores x 2 sequential
elements per core in one NEFF. Params replicated, loaded once.

Algebraic folds (host-side, exact):
  - scores = (Xq Wq + bq)(Xk Wk + bk)^T: the bk cross-term is constant per
    query row -> softmax-invariant, so scores ~ Xq (Wq Wk^T) Xk^T + (bq Wk^T)
    Xk^T. Host precomputes Wqk = Wq@Wk.T -> K projection disappears; the raw
    fp8 Xk^T input doubles as the K operand.
  - ctx@Wo + bo = attnW @ (Xk (Wv Wo) + bv Wo) + bo. Host precomputes
    Wvo = Wv@Wo -> the Wo matmul, ctx transposes, and their psum stages all
    disappear.

Numerics: attention matmuls fp8 DoubleRow with weights pre-scaled (SQ/SV) to
clear the e4m3 subnormal band; scales folded into the exp scale and the LN1
residual add. FFN matmuls bf16 (fp8 noise exceeds the error budget there).

Layout/engines:
  - host pre-transposes Xq/Xk to [D,N] fp8; Q' produced transposed; V'
    natural with a ones column -> softmax denominator rides the ctx matmul
  - residual+LN1 -> bf16 out bounced once through DRAM (read back natural
    for the LN3 residual + DMA-transposed for FFN1)
  - FFN1 evicts psum->hpre on ACT, bn_stats reads psum on DVE, LN+GELU fused
    in one ACT op; h bounced via DRAM with 2-byte DMA-transpose for FFN2
  - LN applies on the Pool engine (SBUF-only ops); rsqrt via DVE bit-trick
    Newton so ACT tables stay {exp, gelu}
"""

import sys

if '/opt/trn_rl_repo' not in sys.path:
    sys.path.insert(0, '/opt/trn_rl_repo')

import math
from contextlib import ExitStack

import numpy as np
import ml_dtypes

import concourse.bass as bass
import concourse.tile as tile
from concourse import bacc, mybir
from concourse import bass2jax
from concourse.masks import make_identity

F32 = mybir.dt.float32
BF16 = mybir.dt.bfloat16
F8 = mybir.dt.float8e4
DR = mybir.MatmulPerfMode.DoubleRow
U32 = mybir.dt.uint32
Alu = mybir.AluOpType
Act = mybir.ActivationFunctionType

EPS = 1e-5
P = 128
SQ = 32.0          # host pre-scale on Wqk (fp8 subnormal fix; keeps the
                   # q' tail well under the e4m3 max of ~224)
SV = 32.0          # host pre-scale on Wvo
SIM_SAFE = False   # swap Gelu->Identity so CoreSim (no gelu) can interpret


# ---------------------------------------------------------------------------
# device program
# ---------------------------------------------------------------------------

def build_program(N=2048, D=768, H=3072, QB=512, n_elems=1, n_reps=1,
                  nontrivial=frozenset()):
    """Build + compile the per-core program (n_elems batch elements).

    nontrivial: subset of {bqk, bv, bo, bf1, bf2, g1b1, gfbf, g2b2} naming
    affine params that are not identity and need real ops emitted. bqk/bv
    arrive pre-folded (bq@Wk.T*SQ, bv@Wo*SV) from the host.
    """
    DC = D // P          # d chunks (6)
    HC = H // P          # h chunks (24)
    RT = N // P          # row tiles (16)
    NB = N // QB         # q blocks (4)
    SB = QB // P         # subtiles per block (4)
    F1N = min(512, H)    # FFN1 n-chunk width
    F1C = H // F1N       # FFN1 n-chunks (6)
    KC = DC // 2         # fp8 DR contraction chunks (3)
    scale = 1.0 / (SQ * math.sqrt(D))

    def slices(total):
        out, lo = [], 0
        while lo < total:
            hi = min(lo + 512, total)
            out.append((lo, hi))
            lo = hi
        return out

    D_SL = slices(D)          # [(0,512),(512,768)]
    D1_SL = slices(D + 1)     # [(0,512),(512,769)]

    nc = bacc.Bacc("TRN2", target_bir_lowering=False, debug=False,
                   num_devices=8)

    # ---- DRAM I/O -----------------------------------------------------
    l_xqT = [nc.dram_tensor(f"xqT{e}", [D, N], F8, kind="ExternalInput")
             for e in range(n_elems)]
    l_xkT = [nc.dram_tensor(f"xkT{e}", [D, N], F8, kind="ExternalInput")
             for e in range(n_elems)]
    l_xqb = [nc.dram_tensor(f"xqb{e}", [N, D], BF16, kind="ExternalInput")
             for e in range(n_elems)]
    d_wqk = nc.dram_tensor("wqk", [D, D], F8, kind="ExternalInput")
    d_wvo = nc.dram_tensor("wvo", [D, D], F8, kind="ExternalInput")
    d_w1 = nc.dram_tensor("w1", [D, H], BF16, kind="ExternalInput")
    d_w2 = nc.dram_tensor("w2", [H, D], BF16, kind="ExternalInput")
    dram_aff = {}
    if "bqk" in nontrivial:
        dram_aff["bqk"] = nc.dram_tensor("bqk", [D], F32,
                                         kind="ExternalInput")
    for nm, sz in (("bv", D), ("bf1", H), ("bf2", D)):
        if nm in nontrivial:
            dram_aff[nm] = nc.dram_tensor(nm, [sz], BF16, kind="ExternalInput")
    if "bo" in nontrivial:
        dram_aff["bo"] = nc.dram_tensor("bo", [D], F32, kind="ExternalInput")
    for nm, sz in (("g1b1", D), ("gfbf", H), ("g2b2", D)):
        if nm in nontrivial:
            dram_aff[nm + "_g"] = nc.dram_tensor(nm + "_g", [sz], F32,
                                                 kind="ExternalInput")
            dram_aff[nm + "_b"] = nc.dram_tensor(nm + "_b", [sz], F32,
                                                 kind="ExternalInput")
    l_y = [nc.dram_tensor(f"y{e}", [N, D], F32, kind="ExternalOutput")
           for e in range(n_elems)]
    # internal scratch
    l_outb = [nc.dram_tensor(f"out_b16_{e}", [N, D], BF16)
              for e in range(n_elems)]
    l_h = [nc.dram_tensor(f"h_b16_{e}", [N, H], BF16)
           for e in range(n_elems)]

    # bn_stats subgroup sizes for D-wide rows
    bn_d = math.gcd(512, D)      # 256 for 768
    bn_dn = D // bn_d

    def emit_rsqrt(pool, nc, var_ap, tag):
        """rstd[P,1] f32 = 1/sqrt(var+EPS), DVE only (no ACT tables)."""
        ve = pool.tile([P, 1], F32, tag=f"rs_ve_{tag}", bufs=2)
        nc.vector.tensor_scalar_add(ve, var_ap, EPS)
        y = pool.tile([P, 1], F32, tag=f"rs_y_{tag}", bufs=2)
        # y_bits = 0x5f3759df - (ve_bits >> 1)  ==  ~(ve_bits>>1) - 0xA0C8A620
        nc.vector.tensor_scalar(
            out=y.bitcast(U32), in0=ve.bitcast(U32),
            scalar1=1, scalar2=0xFFFFFFFF,
            op0=Alu.logical_shift_right, op1=Alu.bitwise_xor)
        nc.vector.tensor_scalar(
            out=y.bitcast(U32), in0=y.bitcast(U32),
            scalar1=0xA0C8A620, scalar2=None, op0=Alu.subtract)
        t = pool.tile([P, 1], F32, tag=f"rs_t_{tag}", bufs=2)
        for _ in range(2):
            nc.vector.tensor_mul(t, y, y)            # y^2
            nc.vector.tensor_mul(t, t, ve)           # v*y^2
            nc.vector.tensor_scalar(out=t, in0=t, scalar1=-0.5, scalar2=1.5,
                                    op0=Alu.mult, op1=Alu.add)
            nc.vector.tensor_mul(y, y, t)
        return y

    def load_rep(pool, nc, dram, sz, tag):
        """Broadcast a [sz] dram vector across partitions -> [P, sz] tile."""
        t = pool.tile([P, sz], dram.dtype, tag=tag, bufs=1)
        ap = dram.ap()
        bcast = bass.AP(tensor=ap.tensor, offset=ap.offset,
                        ap=[[0, P]] + list(ap.ap))
        nc.gpsimd.dma_start(out=t, in_=bcast)
        return t

    with tile.TileContext(nc) as tc, ExitStack() as octx:
        pers = octx.enter_context(tc.tile_pool(name="pers", bufs=1))
        ident = pers.tile([P, P], BF16)
        make_identity(nc, ident)

        # persistent weights; DMA issue order = first-need order, with the
        # big FFN weights issued inside the first element's phase A so the
        # x inputs aren't queued behind them.
        wqk = pers.tile([P, KC, 2, D], F8)
        wvo = pers.tile([P, KC, 2, D], F8)
        w1 = pers.tile([P, DC, H], BF16)
        w2 = pers.tile([P, HC, D], BF16)

        def w_dram(d):
            return d.ap().rearrange("(c i p) f -> p c i f", p=P, i=2)

        for k in range(KC):
            nc.sync.dma_start(out=wqk[:, k], in_=w_dram(d_wqk)[:, k])
        for k in range(KC):
            nc.sync.dma_start(out=wvo[:, k], in_=w_dram(d_wvo)[:, k])

        # optional affine params
        bqk_sb = None
        if "bqk" in nontrivial:
            bqk_sb = pers.tile([P, DC], F32)
            nc.sync.dma_start(out=bqk_sb,
                              in_=dram_aff["bqk"].ap().rearrange(
                                  "(c p) -> p c", p=P))
        ones_row = None
        if any(k in nontrivial for k in ("bv", "bf1", "bf2")):
            ones_row = pers.tile([1, P], BF16)
            nc.vector.memset(ones_row, 1.0)
        bias_rows = {}
        for nm in ("bv", "bf1", "bf2"):
            if nm in nontrivial:
                sz = H if nm == "bf1" else D
                t = pers.tile([1, sz], BF16, tag=f"brow_{nm}")
                nc.sync.dma_start(out=t, in_=dram_aff[nm].ap().rearrange(
                    "(o f) -> o f", o=1))
                bias_rows[nm] = t
        bo_rep = None
        if "bo" in nontrivial:
            bo_rep = load_rep(pers, nc, dram_aff["bo"], D, "bo_rep")
        gain_reps = {}
        for nm in ("g1b1", "gfbf", "g2b2"):
            if nm in nontrivial:
                sz = H if nm == "gfbf" else D
                gain_reps[nm + "_g"] = load_rep(pers, nc, dram_aff[nm + "_g"],
                                                sz, f"grep_{nm}")
                gain_reps[nm + "_b"] = load_rep(pers, nc, dram_aff[nm + "_b"],
                                                sz, f"brep_{nm}")

        def add_bias_row(psum_ap, nm, lo, hi):
            nc.tensor.matmul(psum_ap[:, lo:hi], lhsT=ones_row,
                             rhs=bias_rows[nm][:, lo:hi],
                             start=False, stop=True)

        def post_ln_affine(eng, buf, nm):
            if nm in nontrivial:
                eng.tensor_mul(buf, buf, gain_reps[nm + "_g"])
                eng.tensor_add(buf, buf, gain_reps[nm + "_b"])

        ffn_w_loaded = False
        for _rep in range(n_reps):
          for _e in range(n_elems):
            _pe = f"{_rep}_{_e}"
            d_xqT, d_xkT, d_xqb = l_xqT[_e], l_xkT[_e], l_xqb[_e]
            d_y, d_outb, d_h = l_y[_e], l_outb[_e], l_h[_e]

            ectx = ExitStack()
            crossE = ectx.enter_context(tc.tile_pool(name=f"crossE{_pe}",
                                                     bufs=1))
            outT = crossE.tile([P, DC, N], BF16)

            # =============== Phases A+B share the Q/K/V tiles ==============
            ab_ctx = ectx.enter_context(ExitStack())
            qkv = ab_ctx.enter_context(tc.tile_pool(name=f"qkv{_pe}", bufs=1))
            VW = D + 16
            qT = qkv.tile([P, KC, 2, N], F8)
            kT = qkv.tile([P, KC, 2, N], F8)     # raw Xk^T: K operand + V src
            xqTs = qkv.tile([P, KC, 2, N], F8)
            v = qkv.tile([P, RT // 2, 2, VW], F8)

            def qk_slot(t, m, sl):
                return t[:, m // 2, m % 2, sl]

            def x_dram(d):
                return d.ap().rearrange("(c i p) n -> p c i n", p=P, i=2)

            with ExitStack() as ctx:
                psA = ctx.enter_context(tc.tile_pool(name=f"psA{_pe}", bufs=3,
                                                     space="PSUM"))
                psV = ctx.enter_context(tc.tile_pool(name=f"psV{_pe}", bufs=2,
                                                     space="PSUM"))
                nc.sync.dma_start(out=xqTs, in_=x_dram(d_xqT))
                nc.sync.dma_start(out=kT, in_=x_dram(d_xkT))
                if not ffn_w_loaded:
                    # issue the big FFN weight loads behind the x inputs
                    ffn_w_loaded = True
                    for k in range(DC):
                        nc.sync.dma_start(
                            out=w1[:, k, :], in_=d_w1.ap().rearrange(
                                "(c p) f -> p c f", p=P)[:, k, :])
                    for k in range(HC):
                        nc.sync.dma_start(
                            out=w2[:, k, :], in_=d_w2.ap().rearrange(
                                "(c p) f -> p c f", p=P)[:, k, :])

                # softmax-denominator ones column (whole tile, once)
                nc.gpsimd.memset(v[:, :, :, D:D + 1], 1.0)

                # Q' = Xq @ Wqk, produced transposed [D, N] in fp8
                for rb in range(NB):
                    for m in range(DC):
                        ps = psA.tile([P, QB], F32, tag="psA")
                        for k in range(KC):
                            nc.tensor.matmul(
                                ps,
                                lhsT=wqk[:, k, :, m * P:(m + 1) * P],
                                rhs=xqTs[:, k, :, rb * QB:(rb + 1) * QB],
                                start=(k == 0), stop=(k == KC - 1),
                                perf_mode=DR)
                        dsl = qk_slot(qT, m, slice(rb * QB, (rb + 1) * QB))
                        if "bqk" in nontrivial:
                            nc.scalar.activation(
                                out=dsl, in_=ps, func=Act.Identity,
                                bias=bqk_sb[:, m:m + 1])
                        else:
                            nc.vector.tensor_copy(out=dsl, in_=ps)
                    # V' = Xk @ Wvo for this row block
                    for st_ in range(QB // P):
                        rt = rb * (QB // P) + st_
                        ps = psV.tile([P, D], F32, tag="psV")
                        for lo, hi in D_SL:
                            for k in range(KC):
                                nc.tensor.matmul(
                                    ps[:, lo:hi],
                                    lhsT=kT[:, k, :,
                                            rt * P:(rt + 1) * P],
                                    rhs=wvo[:, k, :, lo:hi],
                                    start=(k == 0),
                                    stop=(k == KC - 1 and
                                          "bv" not in nontrivial),
                                    perf_mode=DR)
                            if "bv" in nontrivial:
                                add_bias_row(ps, "bv", lo, hi)
                        nc.scalar.copy(qk_slot(v, rt, slice(0, D)), ps)

            # =============== Phase B: attention + LN1 =======================
            with ExitStack() as ctx:
                pb = ctx.enter_context(tc.tile_pool(name=f"pb{_pe}", bufs=1))
                psS = ctx.enter_context(tc.tile_pool(name=f"psS{_pe}", bufs=2,
                                                     space="PSUM"))
                psC = ctx.enter_context(tc.tile_pool(name=f"psC{_pe}", bufs=3,
                                                     space="PSUM"))

                def ln1_tail(pend):
                    r_t, st, qs = pend
                    mv = pb.tile([P, 2], F32, tag="mv1", bufs=2)
                    nc.vector.bn_aggr(mv, st)
                    rstd = emit_rsqrt(pb, nc, mv[:, 1:2], "ln1")
                    nmr = pb.tile([P, 1], F32, tag="nmr1", bufs=2)
                    nc.vector.tensor_scalar(out=nmr, in0=mv[:, 0:1],
                                            scalar1=rstd, scalar2=-1.0,
                                            op0=Alu.mult, op1=Alu.mult)
                    out_t = pb.tile([P, D], BF16, tag="out", bufs=2)
                    nc.gpsimd.tensor_scalar(out=out_t, in0=r_t,
                                            scalar1=rstd, scalar2=nmr,
                                            op0=Alu.mult, op1=Alu.add)
                    post_ln_affine(nc.gpsimd, out_t, "g1b1")
                    nc.sync.dma_start(out=d_outb.ap()[qs * P:(qs + 1) * P, :],
                                      in_=out_t)
                    if qs % SB == SB - 1:
                        # whole q-block written: DMA-transpose it for FFN1
                        qb_ = qs // SB
                        for j in range(DC):
                            nc.sync.dma_start(
                                out=outT[:, j, qb_ * QB:(qb_ + 1) * QB],
                                in_=d_outb.ap()[qb_ * QB:(qb_ + 1) * QB,
                                                j * P:(j + 1) * P],
                                transpose=True)

                pendB = None
                for qb in range(NB):
                    eT = pb.tile([P, RT // 2, 2, QB], F8, tag="eT", bufs=2)
                    for kt in range(RT):
                        ps = psS.tile([P, QB], F32, tag="psS")
                        for c in range(KC):
                            nc.tensor.matmul(
                                ps, lhsT=kT[:, c, :, kt * P:(kt + 1) * P],
                                rhs=qT[:, c, :, qb * QB:(qb + 1) * QB],
                                start=(c == 0), stop=(c == KC - 1),
                                perf_mode=DR)
                        nc.scalar.activation(
                            out=qk_slot(eT, kt, slice(0, QB)), in_=ps,
                            func=Act.Exp, scale=scale)
                    for s in range(SB):
                        qs = qb * SB + s          # global q subtile
                        ps = psC.tile([P, D + 1], F32, tag="psC")
                        for lo, hi in D1_SL:
                            for t_ in range(RT // 2):
                                nc.tensor.matmul(
                                    ps[:, lo:hi],
                                    lhsT=eT[:, t_, :, s * P:(s + 1) * P],
                                    rhs=v[:, t_, :, lo:hi],
                                    start=(t_ == 0),
                                    stop=(t_ == RT // 2 - 1),
                                    perf_mode=DR)
                        # r = ctx'/(SV*denom) + xq  (ctx' = psC cols 0..D)
                        den_s = pb.tile([P, 1], F32, tag="den", bufs=2)
                        nc.vector.tensor_scalar_mul(den_s, ps[:, D:D + 1],
                                                    float(SV))
                        recip = pb.tile([P, 1], F32, tag="recip", bufs=2)
                        nc.vector.reciprocal(recip, den_s)
                        xq_t = pb.tile([P, D], BF16, tag="xq", bufs=3)
                        nc.sync.dma_start(out=xq_t,
                                          in_=d_xqb.ap()[qs * P:(qs + 1) * P, :])
                        r_t = pb.tile([P, D], F32, tag="r", bufs=2)
                        nc.vector.tensor_scalar_mul(r_t, ps[:, :D], recip)
                        nc.vector.tensor_add(r_t, r_t, xq_t)
                        if "bo" in nontrivial:
                            nc.vector.tensor_add(r_t, r_t, bo_rep)
                        st = pb.tile([P, bn_dn, 6], F32, tag="st1", bufs=2)
                        for g in range(bn_dn):
                            nc.vector.bn_stats(st[:, g, :],
                                               r_t[:, g * bn_d:(g + 1) * bn_d])
                        if pendB is not None:
                            ln1_tail(pendB)
                        pendB = (r_t, st, qs)
                ln1_tail(pendB)

            ab_ctx.close()  # free Q/K/V + attention SBUF before FFN

            # =============== Phase C: FFN + LN2/LN3 =========================
            with ExitStack() as ctx:
                pc = ctx.enter_context(tc.tile_pool(name=f"pc{_pe}", bufs=1))
                psH = ctx.enter_context(tc.tile_pool(name=f"psH{_pe}", bufs=6,
                                                     space="PSUM"))
                psF = ctx.enter_context(tc.tile_pool(name=f"psF{_pe}", bufs=1,
                                                     space="PSUM"))

                def ln2_tail(pend):
                    hpre, st2, t = pend
                    mv = pc.tile([P, 2], F32, tag="mv2", bufs=2)
                    nc.vector.bn_aggr(mv, st2)
                    rstd = emit_rsqrt(pc, nc, mv[:, 1:2], "ln2")
                    nmr = pc.tile([P, 1], F32, tag="nmr2", bufs=2)
                    nc.vector.tensor_scalar(out=nmr, in0=mv[:, 0:1], scalar1=rstd,
                                            scalar2=-1.0, op0=Alu.mult,
                                            op1=Alu.mult)
                    h_t = pc.tile([P, H], BF16, tag="h", bufs=2)
                    if "gfbf" in nontrivial:
                        tmp = pc.tile([P, H], F32, tag="lnh", bufs=2)
                        nc.vector.tensor_scalar(out=tmp, in0=hpre, scalar1=rstd,
                                                scalar2=nmr, op0=Alu.mult,
                                                op1=Alu.add)
                        post_ln_affine(nc.vector, tmp, "gfbf")
                        nc.scalar.activation(out=h_t, in_=tmp, func=Act.Gelu)
                    else:
                        # fused LN + gelu: gelu(x*rstd + (-mu*rstd))
                        nc.scalar.activation(
                            out=h_t, in_=hpre,
                            func=Act.Identity if SIM_SAFE else Act.Gelu,
                            bias=nmr, scale=rstd)
                    nc.sync.dma_start(out=d_h.ap()[t * P:(t + 1) * P, :], in_=h_t)

                pendC = None
                for t in range(RT):
                    hpre = pc.tile([P, H], BF16, tag="hpre", bufs=2)
                    st2 = pc.tile([P, F1C, 6], F32, tag="st2", bufs=2)
                    for n in range(F1C):
                        ps = psH.tile([P, F1N], F32, tag="psH")
                        for j in range(DC):
                            nc.tensor.matmul(
                                ps, lhsT=outT[:, j, t * P:(t + 1) * P],
                                rhs=w1[:, j, n * F1N:(n + 1) * F1N],
                                start=(j == 0),
                                stop=(j == DC - 1 and "bf1" not in nontrivial))
                        if "bf1" in nontrivial:
                            add_bias_row(ps, "bf1", n * F1N, (n + 1) * F1N)
                        if n % 2 == 0:
                            nc.scalar.copy(hpre[:, n * F1N:(n + 1) * F1N], ps)
                        else:
                            nc.vector.tensor_copy(
                                out=hpre[:, n * F1N:(n + 1) * F1N], in_=ps)
                        nc.vector.bn_stats(st2[:, n, :],
                                           hpre[:, n * F1N:(n + 1) * F1N])
                    if pendC is not None:
                        ln2_tail(pendC)
                    pendC = (hpre, st2, t)
                ln2_tail(pendC)

                def ln3_tail(pend):
                    r2, st3, qs = pend
                    mv3 = pc.tile([P, 2], F32, tag="mv3", bufs=2)
                    nc.vector.bn_aggr(mv3, st3)
                    rstd3 = emit_rsqrt(pc, nc, mv3[:, 1:2], "ln3")
                    nmr3 = pc.tile([P, 1], F32, tag="nmr3", bufs=2)
                    nc.vector.tensor_scalar(out=nmr3, in0=mv3[:, 0:1],
                                            scalar1=rstd3, scalar2=-1.0,
                                            op0=Alu.mult, op1=Alu.mult)
                    y_t = pc.tile([P, D], F32, tag="y", bufs=3)
                    nc.gpsimd.tensor_scalar(out=y_t, in0=r2, scalar1=rstd3,
                                            scalar2=nmr3, op0=Alu.mult,
                                            op1=Alu.add)
                    post_ln_affine(nc.gpsimd, y_t, "g2b2")
                    nc.sync.dma_start(out=d_y.ap()[qs * P:(qs + 1) * P, :],
                                      in_=y_t)

                pendF = None
                for qb in range(NB):
                    hT = pc.tile([P, HC, QB], BF16, tag="hT", bufs=2)
                    for hc in range(HC):
                        nc.sync.dma_start(
                            out=hT[:, hc, :],
                            in_=d_h.ap()[qb * QB:(qb + 1) * QB,
                                         hc * P:(hc + 1) * P],
                            transpose=True)
                    for s in range(SB):
                        qs = qb * SB + s
                        ps = psF.tile([P, D], F32, tag="psF")
                        for lo, hi in D_SL:
                            for hc in range(HC):
                                nc.tensor.matmul(
                                    ps[:, lo:hi],
                                    lhsT=hT[:, hc, s * P:(s + 1) * P],
                                    rhs=w2[:, hc, lo:hi], start=(hc == 0),
                                    stop=(hc == HC - 1 and
                                          "bf2" not in nontrivial))
                            if "bf2" in nontrivial:
                                add_bias_row(ps, "bf2", lo, hi)
                        o_t = pc.tile([P, D], BF16, tag="oldout", bufs=3)
                        nc.sync.dma_start(out=o_t,
                                          in_=d_outb.ap()[qs * P:(qs + 1) * P, :])
                        r2 = pc.tile([P, D], F32, tag="r2", bufs=2)
                        nc.vector.tensor_add(r2, ps, o_t)
                        st3 = pc.tile([P, bn_dn, 6], F32, tag="st3", bufs=2)
                        for g in range(bn_dn):
                            nc.vector.bn_stats(st3[:, g, :],
                                               r2[:, g * bn_d:(g + 1) * bn_d])
                        if pendF is not None:
                            ln3_tail(pendF)
                        pendF = (r2, st3, qs)
                ln3_tail(pendF)

            ectx.close()
    nc.compile()
    return nc


# ---------------------------------------------------------------------------
# SPMD runner (jit once, device-resident buffers)
# ---------------------------------------------------------------------------

class SpmdRunner:
    def __init__(self, nc, n_cores=8):
        import jax
        from jax.sharding import Mesh, PartitionSpec, NamedSharding
        from jax.experimental.shard_map import shard_map
        bass2jax.install_neuronx_cc_hook()
        self.jax = jax
        self.nc = nc
        self.n_cores = n_cores
        in_names, out_names, out_avals, zero_outs = [], [], [], []
        part = nc.partition_id_tensor.name if nc.partition_id_tensor else None
        for alloc in nc.m.functions[0].allocations:
            if not isinstance(alloc, mybir.MemoryLocationSet):
                continue
            name = alloc.memorylocations[0].name
            if alloc.kind == "ExternalInput":
                if name != part:
                    in_names.append(name)
            elif alloc.kind == "ExternalOutput":
                out_names.append(name)
                shape = tuple(alloc.tensor_shape)
                dtype = mybir.dt.np(alloc.dtype)
                out_avals.append(jax.core.ShapedArray(shape, dtype))
                zero_outs.append(np.zeros(shape, dtype))
        self.in_names = in_names
        self.out_names = out_names
        self.out_avals = out_avals
        self.zero_outs = zero_outs
        n_params = len(in_names)
        all_names = in_names + out_names + ([part] if part else [])

        def _body(*args):
            operands = list(args)
            if part is not None:
                operands.append(bass2jax.partition_id_tensor())
            return tuple(bass2jax._bass_exec_p.bind(
                *operands, out_avals=tuple(out_avals),
                in_names=tuple(all_names), out_names=tuple(out_names),
                lowering_input_output_aliases=(),
                sim_require_finite=True, sim_require_nnan=True, nc=nc))

        devices = jax.devices()[:n_cores]
        self.mesh = Mesh(np.asarray(devices), ("core",))
        in_specs = (PartitionSpec("core"),) * (n_params + len(out_names))
        out_specs = (PartitionSpec("core"),) * len(out_names)
        self.fn = jax.jit(
            shard_map(_body, mesh=self.mesh, in_specs=in_specs,
                      out_specs=out_specs, check_rep=False),
            keep_unused=True)
        self.sharding = NamedSharding(self.mesh, PartitionSpec("core"))

    def put_inputs(self, in_maps):
        concat = [np.concatenate([np.asarray(in_maps[c][n])
                                  for c in range(self.n_cores)], axis=0)
                  for n in self.in_names]
        zeros = [np.zeros((self.n_cores * z.shape[0], *z.shape[1:]), z.dtype)
                 for z in self.zero_outs]
        bufs = [self.jax.device_put(a, self.sharding) for a in concat + zeros]
        self.jax.block_until_ready(bufs)
        return bufs

    def run(self, bufs):
        outs = self.fn(*bufs)
        self.jax.block_until_ready(outs)
        return outs

    def results(self, outs):
        res = []
        for c in range(self.n_cores):
            d = {}
            for i, name in enumerate(self.out_names):
                d[name] = np.asarray(outs[i]).reshape(
                    self.n_cores, *self.out_avals[i].shape)[c]
            res.append(d)
        return res


# ---------------------------------------------------------------------------
# host entry point
# ---------------------------------------------------------------------------

_CACHE = {}


def _get_runner(nontrivial, n_elems=2):
    key = (frozenset(nontrivial), n_elems)
    if key not in _CACHE:
        nc = build_program(nontrivial=key[0], n_elems=n_elems)
        _CACHE[key] = SpmdRunner(nc, 8)
    return _CACHE[key]


def _bf16(a):
    return np.asarray(a, dtype=ml_dtypes.bfloat16)


def _f8(a):
    return np.asarray(a, dtype=ml_dtypes.float8_e4m3)


def detect_nontrivial(bq, bk, bv, bo, bf1, bf2, g1, b1, gf, bf, g2, b2):
    nontrivial = set()
    if not (np.allclose(np.asarray(bq), 0.0) and
            np.allclose(np.asarray(bk), 0.0)):
        nontrivial.add("bqk")
    for nm, val in (("bv", bv), ("bo", bo), ("bf1", bf1), ("bf2", bf2)):
        if not np.allclose(np.asarray(val), 0.0):
            nontrivial.add(nm)
    for nm, g_, b_ in (("g1b1", g1, b1), ("gfbf", gf, bf), ("g2b2", g2, b2)):
        if not (np.allclose(np.asarray(g_), 1.0) and
                np.allclose(np.asarray(b_), 0.0)):
            nontrivial.add(nm)
    return nontrivial


def make_in_maps(inputs, nontrivial, n_cores=8, n_elems=2):
    Xq = np.asarray(inputs["query_modal"], np.float32)
    Xk = np.asarray(inputs["key_modal"], np.float32)
    Wq = np.asarray(inputs["Wq"], np.float32)
    Wk = np.asarray(inputs["Wk"], np.float32)
    Wv = np.asarray(inputs["Wv"], np.float32)
    Wo = np.asarray(inputs["Wo"], np.float32)
    weights = {
        "wqk": _f8((Wq @ Wk.T) * SQ),
        "wvo": _f8((Wv @ Wo) * SV),
        "w1": _bf16(inputs["W1"]), "w2": _bf16(inputs["W2"]),
    }
    if "bqk" in nontrivial:
        # folded: scores row-bias = bq @ Wk^T (the bk term is softmax-inv)
        weights["bqk"] = np.asarray(
            np.asarray(inputs["bq"], np.float32) @ Wk.T * SQ, np.float32)
    if "bv" in nontrivial:
        weights["bv"] = _bf16(
            (np.asarray(inputs["bv"], np.float32) @ Wo) * SV)
    if "bo" in nontrivial:
        weights["bo"] = np.asarray(inputs["bo"], np.float32)
    for nm in ("bf1", "bf2"):
        if nm in nontrivial:
            weights[nm] = _bf16(np.float32(inputs[nm]))
    for nm, (gk, bk_) in (("g1b1", ("g1", "b1")), ("gfbf", ("gf", "bf")),
                          ("g2b2", ("g2", "b2"))):
        if nm in nontrivial:
            weights[nm + "_g"] = np.asarray(inputs[gk], np.float32)
            weights[nm + "_b"] = np.asarray(inputs[bk_], np.float32)

    in_maps = []
    for c in range(n_cores):
        m = dict(weights)
        for e in range(n_elems):
            b = e * n_cores + c
            m[f"xqT{e}"] = _f8(np.ascontiguousarray(Xq[b].T))
            m[f"xkT{e}"] = _f8(np.ascontiguousarray(Xk[b].T))
            m[f"xqb{e}"] = _bf16(Xq[b])
        in_maps.append(m)
    return in_maps


def kernel(query_modal, key_modal, Wq, bq, Wk, bk, Wv, bv, Wo, bo,
           g1, b1, W1, bf1, gf, bf, W2, bf2, g2, b2):
    query_modal = np.asarray(query_modal, np.float32)
    B, N, D = query_modal.shape

    nontrivial = detect_nontrivial(bq, bk, bv, bo, bf1, bf2,
                                   g1, b1, gf, bf, g2, b2)
    n_cores = 8
    n_elems = (B + n_cores - 1) // n_cores
    runner = _get_runner(nontrivial, n_elems)

    inputs = dict(query_modal=query_modal, key_modal=key_modal,
                  Wq=Wq, bq=bq, Wk=Wk, bk=bk, Wv=Wv, bv=bv, Wo=Wo, bo=bo,
                  g1=g1, b1=b1, W1=W1, bf1=bf1, gf=gf, bf=bf,
                  W2=W2, bf2=bf2, g2=g2, b2=b2)
    in_maps = make_in_maps(inputs, nontrivial, n_cores, n_elems)
    bufs = runner.put_inputs(in_maps)
    outs = runner.run(bufs)
    res = runner.results(outs)
    y = np.empty((B, N, D), np.float32)
    for c in range(n_cores):
        for e in range(n_elems):
            y[e * n_cores + c] = res[c][f"y{e}"]
    return y



# revision 11
# speedup vs baseline: 9.0987x; 9.0987x over previous
"""Trainium2 Bass kernel for CrossModalAttentionImproved.

Single-head cross attention + FFN transformer block:
  q = Xq@Wq+bq; k = Xk@Wk+bk; v = Xk@Wv+bv
  attn = softmax(q k^T / sqrt(D)); ctx = attn@v
  out = LN(Xq + ctx@Wo + bo; g1,b1)
  h = gelu(LN(out@W1 + bf1; gf,bf))
  y = LN(out + h@W2 + bf2; g2,b2)

Sharding: data-parallel over batch. B=16 across 8 cores x 2 sequential
elements per core in one NEFF. Params replicated, loaded once.

Algebraic folds (host-side, exact):
  - scores = (Xq Wq + bq)(Xk Wk + bk)^T: the bk cross-term is constant per
    query row -> softmax-invariant, so scores ~ Xq (Wq Wk^T) Xk^T + (bq Wk^T)
    Xk^T. Host precomputes Wqk = Wq@Wk.T -> K projection disappears; the raw
    fp8 Xk^T input doubles as the K operand.
  - ctx@Wo + bo = attnW @ (Xk (Wv Wo) + bv Wo) + bo. Host precomputes
    Wvo = Wv@Wo -> the Wo matmul, ctx transposes, and their psum stages all
    disappear.

Numerics: attention matmuls fp8 DoubleRow with weights pre-scaled (SQ/SV) to
clear the e4m3 subnormal band; scales folded into the exp scale and the LN1
residual add. FFN matmuls bf16 (fp8 noise exceeds the error budget there).

Layout/engines:
  - host pre-transposes Xq/Xk to [D,N] fp8; Q' produced transposed; V'
    natural with a ones column -> softmax denominator rides the ctx matmul
  - residual+LN1 -> bf16 out bounced once through DRAM (read back natural
    for the LN3 residual + DMA-transposed for FFN1)
  - FFN1 evicts psum->hpre on ACT, bn_stats reads psum on DVE, LN+GELU fused
    in one ACT op; h bounced via DRAM with 2-byte DMA-transpose for FFN2
  - LN applies on the Pool engine (SBUF-only ops); rsqrt via DVE bit-trick
    Newton so ACT tables stay {exp, gelu}
"""

import sys

if '/opt/trn_rl_repo' not in sys.path:
    sys.path.insert(0, '/opt/trn_rl_repo')

import math
from contextlib import ExitStack

import numpy as np
import ml_dtypes

import concourse.bass as bass
import concourse.tile as tile
from concourse import bacc, mybir
from concourse import bass2jax
from concourse.masks import make_identity

F32 = mybir.dt.float32
BF16 = mybir.dt.bfloat16
F8 = mybir.dt.float8e4
DR = mybir.MatmulPerfMode.DoubleRow
U32 = mybir.dt.uint32
Alu = mybir.AluOpType
Act = mybir.ActivationFunctionType

EPS = 1e-5
P = 128
SQ = 32.0          # host pre-scale on Wqk (fp8 subnormal fix; keeps the
                   # q' tail well under the e4m3 max of ~224)
SV = 32.0          # host pre-scale on Wvo
SIM_SAFE = False   # swap Gelu->Identity so CoreSim (no gelu) can interpret


# ---------------------------------------------------------------------------
# device program
# ---------------------------------------------------------------------------

def build_program(N=2048, D=768, H=3072, QB=512, n_elems=1, n_reps=1,
                  nontrivial=frozenset()):
    """Build + compile the per-core program (n_elems batch elements).

    nontrivial: subset of {bqk, bv, bo, bf1, bf2, g1b1, gfbf, g2b2} naming
    affine params that are not identity and need real ops emitted. bqk/bv
    arrive pre-folded (bq@Wk.T*SQ, bv@Wo*SV) from the host.
    """
    DC = D // P          # d chunks (6)
    HC = H // P          # h chunks (24)
    RT = N // P          # row tiles (16)
    NB = N // QB         # q blocks (4)
    SB = QB // P         # subtiles per block (4)
    F1N = min(512, H)    # FFN1 n-chunk width
    F1C = H // F1N       # FFN1 n-chunks (6)
    KC = DC // 2         # fp8 DR contraction chunks (3)
    scale = 1.0 / (SQ * math.sqrt(D))

    def slices(total):
        out, lo = [], 0
        while lo < total:
            hi = min(lo + 512, total)
            out.append((lo, hi))
            lo = hi
        return out

    D_SL = slices(D)          # [(0,512),(512,768)]
    D1_SL = slices(D + 1)     # [(0,512),(512,769)]

    nc = bacc.Bacc("TRN2", target_bir_lowering=False, debug=False,
                   num_devices=8)

    # ---- DRAM I/O -----------------------------------------------------
    # Per-call dispatch overhead is ~180us per dram I/O tensor (axon PJRT),
    # so all inputs are packed into one blob per dtype and carved into
    # views via manual APs.
    f8_len = n_elems * 2 * D * N + 2 * D * D
    bf_len = n_elems * N * D + D * H + H * D
    d_in = nc.dram_tensor("in_u8", [f8_len + 2 * bf_len], mybir.dt.uint8,
                          kind="ExternalInput")

    def carve(byte_off, dims, dt):
        total = mybir.dt.size(dt)
        for n in dims:
            total *= n
        flat = bass.AP(tensor=d_in.ap().tensor, offset=byte_off,
                       ap=[[1, total]]).bitcast(dt)
        if len(dims) == 1:
            return flat
        assert len(dims) == 2
        return flat.rearrange("(a b) -> a b", a=dims[0])

    BO_BF = f8_len  # byte offset of the bf16 region
    l_xqT = [carve(e * 2 * D * N, [D, N], F8) for e in range(n_elems)]
    l_xkT = [carve(e * 2 * D * N + D * N, [D, N], F8)
             for e in range(n_elems)]
    a_wqk = carve(n_elems * 2 * D * N, [D, D], F8)
    a_wvo = carve(n_elems * 2 * D * N + D * D, [D, D], F8)
    l_xqb = [carve(BO_BF + 2 * e * N * D, [N, D], BF16)
             for e in range(n_elems)]
    a_w1 = carve(BO_BF + 2 * n_elems * N * D, [D, H], BF16)
    a_w2 = carve(BO_BF + 2 * (n_elems * N * D + D * H), [H, D], BF16)
    dram_aff = {}
    if "bqk" in nontrivial:
        dram_aff["bqk"] = nc.dram_tensor("bqk", [D], F32,
                                         kind="ExternalInput")
    for nm, sz in (("bv", D), ("bf1", H), ("bf2", D)):
        if nm in nontrivial:
            dram_aff[nm] = nc.dram_tensor(nm, [sz], BF16, kind="ExternalInput")
    if "bo" in nontrivial:
        dram_aff["bo"] = nc.dram_tensor("bo", [D], F32, kind="ExternalInput")
    for nm, sz in (("g1b1", D), ("gfbf", H), ("g2b2", D)):
        if nm in nontrivial:
            dram_aff[nm + "_g"] = nc.dram_tensor(nm + "_g", [sz], F32,
                                                 kind="ExternalInput")
            dram_aff[nm + "_b"] = nc.dram_tensor(nm + "_b", [sz], F32,
                                                 kind="ExternalInput")
    d_y = nc.dram_tensor("y", [n_elems * N, D], F32, kind="ExternalOutput")
    l_y = [d_y.ap()[e * N:(e + 1) * N, :] for e in range(n_elems)]
    # internal scratch
    l_outb = [nc.dram_tensor(f"out_b16_{e}", [N, D], BF16)
              for e in range(n_elems)]
    l_h = [nc.dram_tensor(f"h_b16_{e}", [N, H], BF16)
           for e in range(n_elems)]

    # bn_stats subgroup sizes for D-wide rows
    bn_d = math.gcd(512, D)      # 256 for 768
    bn_dn = D // bn_d

    def emit_rsqrt(pool, nc, var_ap, tag):
        """rstd[P,1] f32 = 1/sqrt(var+EPS), DVE only (no ACT tables)."""
        ve = pool.tile([P, 1], F32, tag=f"rs_ve_{tag}", bufs=2)
        nc.vector.tensor_scalar_add(ve, var_ap, EPS)
        y = pool.tile([P, 1], F32, tag=f"rs_y_{tag}", bufs=2)
        # y_bits = 0x5f3759df - (ve_bits >> 1)  ==  ~(ve_bits>>1) - 0xA0C8A620
        nc.vector.tensor_scalar(
            out=y.bitcast(U32), in0=ve.bitcast(U32),
            scalar1=1, scalar2=0xFFFFFFFF,
            op0=Alu.logical_shift_right, op1=Alu.bitwise_xor)
        nc.vector.tensor_scalar(
            out=y.bitcast(U32), in0=y.bitcast(U32),
            scalar1=0xA0C8A620, scalar2=None, op0=Alu.subtract)
        t = pool.tile([P, 1], F32, tag=f"rs_t_{tag}", bufs=2)
        for _ in range(2):
            nc.vector.tensor_mul(t, y, y)            # y^2
            nc.vector.tensor_mul(t, t, ve)           # v*y^2
            nc.vector.tensor_scalar(out=t, in0=t, scalar1=-0.5, scalar2=1.5,
                                    op0=Alu.mult, op1=Alu.add)
            nc.vector.tensor_mul(y, y, t)
        return y

    def load_rep(pool, nc, dram, sz, tag):
        """Broadcast a [sz] dram vector across partitions -> [P, sz] tile."""
        t = pool.tile([P, sz], dram.dtype, tag=tag, bufs=1)
        ap = dram.ap()
        bcast = bass.AP(tensor=ap.tensor, offset=ap.offset,
                        ap=[[0, P]] + list(ap.ap))
        nc.gpsimd.dma_start(out=t, in_=bcast)
        return t

    with tile.TileContext(nc) as tc, ExitStack() as octx:
        pers = octx.enter_context(tc.tile_pool(name="pers", bufs=1))
        ident = pers.tile([P, P], BF16)
        make_identity(nc, ident)

        # persistent weights; DMA issue order = first-need order, with the
        # big FFN weights issued inside the first element's phase A so the
        # x inputs aren't queued behind them.
        wqk = pers.tile([P, KC, 2, D], F8)
        wvo = pers.tile([P, KC, 2, D], F8)
        w1 = pers.tile([P, DC, H], BF16)
        w2 = pers.tile([P, HC, D], BF16)

        def w_dram(a):
            return a.rearrange("(c i p) f -> p c i f", p=P, i=2)

        for k in range(KC):
            nc.sync.dma_start(out=wqk[:, k], in_=w_dram(a_wqk)[:, k])
        for k in range(KC):
            nc.sync.dma_start(out=wvo[:, k], in_=w_dram(a_wvo)[:, k])

        # optional affine params
        bqk_sb = None
        if "bqk" in nontrivial:
            bqk_sb = pers.tile([P, DC], F32)
            nc.sync.dma_start(out=bqk_sb,
                              in_=dram_aff["bqk"].ap().rearrange(
                                  "(c p) -> p c", p=P))
        ones_row = None
        if any(k in nontrivial for k in ("bv", "bf1", "bf2")):
            ones_row = pers.tile([1, P], BF16)
            nc.vector.memset(ones_row, 1.0)
        bias_rows = {}
        for nm in ("bv", "bf1", "bf2"):
            if nm in nontrivial:
                sz = H if nm == "bf1" else D
                t = pers.tile([1, sz], BF16, tag=f"brow_{nm}")
                nc.sync.dma_start(out=t, in_=dram_aff[nm].ap().rearrange(
                    "(o f) -> o f", o=1))
                bias_rows[nm] = t
        bo_rep = None
        if "bo" in nontrivial:
            bo_rep = load_rep(pers, nc, dram_aff["bo"], D, "bo_rep")
        gain_reps = {}
        for nm in ("g1b1", "gfbf", "g2b2"):
            if nm in nontrivial:
                sz = H if nm == "gfbf" else D
                gain_reps[nm + "_g"] = load_rep(pers, nc, dram_aff[nm + "_g"],
                                                sz, f"grep_{nm}")
                gain_reps[nm + "_b"] = load_rep(pers, nc, dram_aff[nm + "_b"],
                                                sz, f"brep_{nm}")

        def add_bias_row(psum_ap, nm, lo, hi):
            nc.tensor.matmul(psum_ap[:, lo:hi], lhsT=ones_row,
                             rhs=bias_rows[nm][:, lo:hi],
                             start=False, stop=True)

        def post_ln_affine(eng, buf, nm):
            if nm in nontrivial:
                eng.tensor_mul(buf, buf, gain_reps[nm + "_g"])
                eng.tensor_add(buf, buf, gain_reps[nm + "_b"])

        ffn_w_loaded = False
        for _rep in range(n_reps):
          for _e in range(n_elems):
            _pe = f"{_rep}_{_e}"
            d_xqT, d_xkT, d_xqb = l_xqT[_e], l_xkT[_e], l_xqb[_e]
            d_y, d_outb, d_h = l_y[_e], l_outb[_e], l_h[_e]

            ectx = ExitStack()
            crossE = ectx.enter_context(tc.tile_pool(name=f"crossE{_pe}",
                                                     bufs=1))
            outT = crossE.tile([P, DC, N], BF16)

            # =============== Phases A+B share the Q/K/V tiles ==============
            ab_ctx = ectx.enter_context(ExitStack())
            qkv = ab_ctx.enter_context(tc.tile_pool(name=f"qkv{_pe}", bufs=1))
            VW = D + 16
            qT = qkv.tile([P, KC, 2, N], F8)
            kT = qkv.tile([P, KC, 2, N], F8)     # raw Xk^T: K operand + V src
            xqTs = qkv.tile([P, KC, 2, N], F8)
            v = qkv.tile([P, RT // 2, 2, VW], F8)

            def qk_slot(t, m, sl):
                return t[:, m // 2, m % 2, sl]

            def x_dram(a):
                return a.rearrange("(c i p) n -> p c i n", p=P, i=2)

            with ExitStack() as ctx:
                psA = ctx.enter_context(tc.tile_pool(name=f"psA{_pe}", bufs=3,
                                                     space="PSUM"))
                psV = ctx.enter_context(tc.tile_pool(name=f"psV{_pe}", bufs=2,
                                                     space="PSUM"))
                nc.sync.dma_start(out=xqTs, in_=x_dram(d_xqT))
                nc.sync.dma_start(out=kT, in_=x_dram(d_xkT))
                if not ffn_w_loaded:
                    # issue the big FFN weight loads behind the x inputs
                    ffn_w_loaded = True
                    for k in range(DC):
                        nc.sync.dma_start(
                            out=w1[:, k, :], in_=a_w1.rearrange(
                                "(c p) f -> p c f", p=P)[:, k, :])
                    for k in range(HC):
                        nc.sync.dma_start(
                            out=w2[:, k, :], in_=a_w2.rearrange(
                                "(c p) f -> p c f", p=P)[:, k, :])

                # softmax-denominator ones column (whole tile, once)
                nc.gpsimd.memset(v[:, :, :, D:D + 1], 1.0)

                # Q' = Xq @ Wqk, produced transposed [D, N] in fp8
                for rb in range(NB):
                    for m in range(DC):
                        ps = psA.tile([P, QB], F32, tag="psA")
                        for k in range(KC):
                            nc.tensor.matmul(
                                ps,
                                lhsT=wqk[:, k, :, m * P:(m + 1) * P],
                                rhs=xqTs[:, k, :, rb * QB:(rb + 1) * QB],
                                start=(k == 0), stop=(k == KC - 1),
                                perf_mode=DR)
                        dsl = qk_slot(qT, m, slice(rb * QB, (rb + 1) * QB))
                        if "bqk" in nontrivial:
                            nc.scalar.activation(
                                out=dsl, in_=ps, func=Act.Identity,
                                bias=bqk_sb[:, m:m + 1])
                        else:
                            nc.vector.tensor_copy(out=dsl, in_=ps)
                    # V' = Xk @ Wvo for this row block
                    for st_ in range(QB // P):
                        rt = rb * (QB // P) + st_
                        ps = psV.tile([P, D], F32, tag="psV")
                        for lo, hi in D_SL:
                            for k in range(KC):
                                nc.tensor.matmul(
                                    ps[:, lo:hi],
                                    lhsT=kT[:, k, :,
                                            rt * P:(rt + 1) * P],
                                    rhs=wvo[:, k, :, lo:hi],
                                    start=(k == 0),
                                    stop=(k == KC - 1 and
                                          "bv" not in nontrivial),
                                    perf_mode=DR)
                            if "bv" in nontrivial:
                                add_bias_row(ps, "bv", lo, hi)
                        nc.scalar.copy(qk_slot(v, rt, slice(0, D)), ps)

            # =============== Phase B: attention + LN1 =======================
            with ExitStack() as ctx:
                pb = ctx.enter_context(tc.tile_pool(name=f"pb{_pe}", bufs=1))
                psS = ctx.enter_context(tc.tile_pool(name=f"psS{_pe}", bufs=2,
                                                     space="PSUM"))
                psC = ctx.enter_context(tc.tile_pool(name=f"psC{_pe}", bufs=3,
                                                     space="PSUM"))

                def ln1_tail(pend):
                    r_t, st, qs = pend
                    mv = pb.tile([P, 2], F32, tag="mv1", bufs=2)
                    nc.vector.bn_aggr(mv, st)
                    rstd = emit_rsqrt(pb, nc, mv[:, 1:2], "ln1")
                    nmr = pb.tile([P, 1], F32, tag="nmr1", bufs=2)
                    nc.vector.tensor_scalar(out=nmr, in0=mv[:, 0:1],
                                            scalar1=rstd, scalar2=-1.0,
                                            op0=Alu.mult, op1=Alu.mult)
                    out_t = pb.tile([P, D], BF16, tag="out", bufs=2)
                    nc.gpsimd.tensor_scalar(out=out_t, in0=r_t,
                                            scalar1=rstd, scalar2=nmr,
                                            op0=Alu.mult, op1=Alu.add)
                    post_ln_affine(nc.gpsimd, out_t, "g1b1")
                    nc.sync.dma_start(out=d_outb.ap()[qs * P:(qs + 1) * P, :],
                                      in_=out_t)
                    if qs % SB == SB - 1:
                        # whole q-block written: DMA-transpose it for FFN1
                        qb_ = qs // SB
                        for j in range(DC):
                            nc.sync.dma_start(
                                out=outT[:, j, qb_ * QB:(qb_ + 1) * QB],
                                in_=d_outb.ap()[qb_ * QB:(qb_ + 1) * QB,
                                                j * P:(j + 1) * P],
                                transpose=True)

                pendB = None
                for qb in range(NB):
                    eT = pb.tile([P, RT // 2, 2, QB], F8, tag="eT", bufs=2)
                    for kt in range(RT):
                        ps = psS.tile([P, QB], F32, tag="psS")
                        for c in range(KC):
                            nc.tensor.matmul(
                                ps, lhsT=kT[:, c, :, kt * P:(kt + 1) * P],
                                rhs=qT[:, c, :, qb * QB:(qb + 1) * QB],
                                start=(c == 0), stop=(c == KC - 1),
                                perf_mode=DR)
                        nc.scalar.activation(
                            out=qk_slot(eT, kt, slice(0, QB)), in_=ps,
                            func=Act.Exp, scale=scale)
                    for s in range(SB):
                        qs = qb * SB + s          # global q subtile
                        ps = psC.tile([P, D + 1], F32, tag="psC")
                        for lo, hi in D1_SL:
                            for t_ in range(RT // 2):
                                nc.tensor.matmul(
                                    ps[:, lo:hi],
                                    lhsT=eT[:, t_, :, s * P:(s + 1) * P],
                                    rhs=v[:, t_, :, lo:hi],
                                    start=(t_ == 0),
                                    stop=(t_ == RT // 2 - 1),
                                    perf_mode=DR)
                        # r = ctx'/(SV*denom) + xq  (ctx' = psC cols 0..D)
                        den_s = pb.tile([P, 1], F32, tag="den", bufs=2)
                        nc.vector.tensor_scalar_mul(den_s, ps[:, D:D + 1],
                                                    float(SV))
                        recip = pb.tile([P, 1], F32, tag="recip", bufs=2)
                        nc.vector.reciprocal(recip, den_s)
                        xq_t = pb.tile([P, D], BF16, tag="xq", bufs=3)
                        nc.sync.dma_start(out=xq_t,
                                          in_=d_xqb[qs * P:(qs + 1) * P, :])
                        r_t = pb.tile([P, D], F32, tag="r", bufs=2)
                        nc.vector.tensor_scalar_mul(r_t, ps[:, :D], recip)
                        nc.vector.tensor_add(r_t, r_t, xq_t)
                        if "bo" in nontrivial:
                            nc.vector.tensor_add(r_t, r_t, bo_rep)
                        st = pb.tile([P, bn_dn, 6], F32, tag="st1", bufs=2)
                        for g in range(bn_dn):
                            nc.vector.bn_stats(st[:, g, :],
                                               r_t[:, g * bn_d:(g + 1) * bn_d])
                        if pendB is not None:
                            ln1_tail(pendB)
                        pendB = (r_t, st, qs)
                ln1_tail(pendB)

            ab_ctx.close()  # free Q/K/V + attention SBUF before FFN

            # =============== Phase C: FFN + LN2/LN3 =========================
            with ExitStack() as ctx:
                pc = ctx.enter_context(tc.tile_pool(name=f"pc{_pe}", bufs=1))
                psH = ctx.enter_context(tc.tile_pool(name=f"psH{_pe}", bufs=6,
                                                     space="PSUM"))
                psF = ctx.enter_context(tc.tile_pool(name=f"psF{_pe}", bufs=1,
                                                     space="PSUM"))

                def ln2_tail(pend):
                    hpre, st2, t = pend
                    mv = pc.tile([P, 2], F32, tag="mv2", bufs=2)
                    nc.vector.bn_aggr(mv, st2)
                    rstd = emit_rsqrt(pc, nc, mv[:, 1:2], "ln2")
                    nmr = pc.tile([P, 1], F32, tag="nmr2", bufs=2)
                    nc.vector.tensor_scalar(out=nmr, in0=mv[:, 0:1], scalar1=rstd,
                                            scalar2=-1.0, op0=Alu.mult,
                                            op1=Alu.mult)
                    h_t = pc.tile([P, H], BF16, tag="h", bufs=2)
                    if "gfbf" in nontrivial:
                        tmp = pc.tile([P, H], F32, tag="lnh", bufs=2)
                        nc.vector.tensor_scalar(out=tmp, in0=hpre, scalar1=rstd,
                                                scalar2=nmr, op0=Alu.mult,
                                                op1=Alu.add)
                        post_ln_affine(nc.vector, tmp, "gfbf")
                        nc.scalar.activation(out=h_t, in_=tmp, func=Act.Gelu)
                    else:
                        # fused LN + gelu: gelu(x*rstd + (-mu*rstd))
                        nc.scalar.activation(
                            out=h_t, in_=hpre,
                            func=Act.Identity if SIM_SAFE else Act.Gelu,
                            bias=nmr, scale=rstd)
                    nc.sync.dma_start(out=d_h.ap()[t * P:(t + 1) * P, :], in_=h_t)

                pendC = None
                for t in range(RT):
                    hpre = pc.tile([P, H], BF16, tag="hpre", bufs=2)
                    st2 = pc.tile([P, F1C, 6], F32, tag="st2", bufs=2)
                    for n in range(F1C):
                        ps = psH.tile([P, F1N], F32, tag="psH")
                        for j in range(DC):
                            nc.tensor.matmul(
                                ps, lhsT=outT[:, j, t * P:(t + 1) * P],
                                rhs=w1[:, j, n * F1N:(n + 1) * F1N],
                                start=(j == 0),
                                stop=(j == DC - 1 and "bf1" not in nontrivial))
                        if "bf1" in nontrivial:
                            add_bias_row(ps, "bf1", n * F1N, (n + 1) * F1N)
                        if n % 2 == 0:
                            nc.scalar.copy(hpre[:, n * F1N:(n + 1) * F1N], ps)
                        else:
                            nc.vector.tensor_copy(
                                out=hpre[:, n * F1N:(n + 1) * F1N], in_=ps)
                        nc.vector.bn_stats(st2[:, n, :],
                                           hpre[:, n * F1N:(n + 1) * F1N])
                    if pendC is not None:
                        ln2_tail(pendC)
                    pendC = (hpre, st2, t)
                ln2_tail(pendC)

                def ln3_tail(pend):
                    r2, st3, qs = pend
                    mv3 = pc.tile([P, 2], F32, tag="mv3", bufs=2)
                    nc.vector.bn_aggr(mv3, st3)
                    rstd3 = emit_rsqrt(pc, nc, mv3[:, 1:2], "ln3")
                    nmr3 = pc.tile([P, 1], F32, tag="nmr3", bufs=2)
                    nc.vector.tensor_scalar(out=nmr3, in0=mv3[:, 0:1],
                                            scalar1=rstd3, scalar2=-1.0,
                                            op0=Alu.mult, op1=Alu.mult)
                    y_t = pc.tile([P, D], F32, tag="y", bufs=3)
                    nc.gpsimd.tensor_scalar(out=y_t, in0=r2, scalar1=rstd3,
                                            scalar2=nmr3, op0=Alu.mult,
                                            op1=Alu.add)
                    post_ln_affine(nc.gpsimd, y_t, "g2b2")
                    nc.sync.dma_start(out=d_y[qs * P:(qs + 1) * P, :],
                                      in_=y_t)

                pendF = None
                for qb in range(NB):
                    hT = pc.tile([P, HC, QB], BF16, tag="hT", bufs=2)
                    for hc in range(HC):
                        nc.sync.dma_start(
                            out=hT[:, hc, :],
                            in_=d_h.ap()[qb * QB:(qb + 1) * QB,
                                         hc * P:(hc + 1) * P],
                            transpose=True)
                    for s in range(SB):
                        qs = qb * SB + s
                        ps = psF.tile([P, D], F32, tag="psF")
                        for lo, hi in D_SL:
                            for hc in range(HC):
                                nc.tensor.matmul(
                                    ps[:, lo:hi],
                                    lhsT=hT[:, hc, s * P:(s + 1) * P],
                                    rhs=w2[:, hc, lo:hi], start=(hc == 0),
                                    stop=(hc == HC - 1 and
                                          "bf2" not in nontrivial))
                            if "bf2" in nontrivial:
                                add_bias_row(ps, "bf2", lo, hi)
                        o_t = pc.tile([P, D], BF16, tag="oldout", bufs=3)
                        nc.sync.dma_start(out=o_t,
                                          in_=d_outb.ap()[qs * P:(qs + 1) * P, :])
                        r2 = pc.tile([P, D], F32, tag="r2", bufs=2)
                        nc.vector.tensor_add(r2, ps, o_t)
                        st3 = pc.tile([P, bn_dn, 6], F32, tag="st3", bufs=2)
                        for g in range(bn_dn):
                            nc.vector.bn_stats(st3[:, g, :],
                                               r2[:, g * bn_d:(g + 1) * bn_d])
                        if pendF is not None:
                            ln3_tail(pendF)
                        pendF = (r2, st3, qs)
                ln3_tail(pendF)

            ectx.close()
    nc.compile()
    return nc


# ---------------------------------------------------------------------------
# SPMD runner (jit once, device-resident buffers)
# ---------------------------------------------------------------------------

class SpmdRunner:
    def __init__(self, nc, n_cores=8):
        import jax
        from jax.sharding import Mesh, PartitionSpec, NamedSharding
        from jax.experimental.shard_map import shard_map
        bass2jax.install_neuronx_cc_hook()
        self.jax = jax
        self.nc = nc
        self.n_cores = n_cores
        in_names, out_names, out_avals, zero_outs = [], [], [], []
        part = nc.partition_id_tensor.name if nc.partition_id_tensor else None
        for alloc in nc.m.functions[0].allocations:
            if not isinstance(alloc, mybir.MemoryLocationSet):
                continue
            name = alloc.memorylocations[0].name
            if alloc.kind == "ExternalInput":
                if name != part:
                    in_names.append(name)
            elif alloc.kind == "ExternalOutput":
                out_names.append(name)
                shape = tuple(alloc.tensor_shape)
                dtype = mybir.dt.np(alloc.dtype)
                out_avals.append(jax.core.ShapedArray(shape, dtype))
                zero_outs.append(np.zeros(shape, dtype))
        self.in_names = in_names
        self.out_names = out_names
        self.out_avals = out_avals
        self.zero_outs = zero_outs
        n_params = len(in_names)
        all_names = in_names + out_names + ([part] if part else [])

        def _body(*args):
            operands = list(args)
            if part is not None:
                operands.append(bass2jax.partition_id_tensor())
            return tuple(bass2jax._bass_exec_p.bind(
                *operands, out_avals=tuple(out_avals),
                in_names=tuple(all_names), out_names=tuple(out_names),
                lowering_input_output_aliases=(),
                sim_require_finite=True, sim_require_nnan=True, nc=nc))

        devices = jax.devices()[:n_cores]
        self.mesh = Mesh(np.asarray(devices), ("core",))
        in_specs = (PartitionSpec("core"),) * (n_params + len(out_names))
        out_specs = (PartitionSpec("core"),) * len(out_names)
        self.fn = jax.jit(
            shard_map(_body, mesh=self.mesh, in_specs=in_specs,
                      out_specs=out_specs, check_rep=False),
            keep_unused=True)
        self.sharding = NamedSharding(self.mesh, PartitionSpec("core"))

    def put_inputs(self, in_maps):
        concat = [np.concatenate([np.asarray(in_maps[c][n])
                                  for c in range(self.n_cores)], axis=0)
                  for n in self.in_names]
        zeros = [np.zeros((self.n_cores * z.shape[0], *z.shape[1:]), z.dtype)
                 for z in self.zero_outs]
        bufs = [self.jax.device_put(a, self.sharding) for a in concat + zeros]
        self.jax.block_until_ready(bufs)
        return bufs

    def run(self, bufs):
        outs = self.fn(*bufs)
        self.jax.block_until_ready(outs)
        return outs

    def results(self, outs):
        res = []
        for c in range(self.n_cores):
            d = {}
            for i, name in enumerate(self.out_names):
                d[name] = np.asarray(outs[i]).reshape(
                    self.n_cores, *self.out_avals[i].shape)[c]
            res.append(d)
        return res


# ---------------------------------------------------------------------------
# host entry point
# ---------------------------------------------------------------------------

_CACHE = {}


def _get_runner(nontrivial, n_elems=2):
    key = (frozenset(nontrivial), n_elems)
    if key not in _CACHE:
        nc = build_program(nontrivial=key[0], n_elems=n_elems)
        _CACHE[key] = SpmdRunner(nc, 8)
    return _CACHE[key]


def _bf16(a):
    return np.asarray(a, dtype=ml_dtypes.bfloat16)


def _f8(a):
    return np.asarray(a, dtype=ml_dtypes.float8_e4m3)


def detect_nontrivial(bq, bk, bv, bo, bf1, bf2, g1, b1, gf, bf, g2, b2):
    nontrivial = set()
    if not (np.allclose(np.asarray(bq), 0.0) and
            np.allclose(np.asarray(bk), 0.0)):
        nontrivial.add("bqk")
    for nm, val in (("bv", bv), ("bo", bo), ("bf1", bf1), ("bf2", bf2)):
        if not np.allclose(np.asarray(val), 0.0):
            nontrivial.add(nm)
    for nm, g_, b_ in (("g1b1", g1, b1), ("gfbf", gf, bf), ("g2b2", g2, b2)):
        if not (np.allclose(np.asarray(g_), 1.0) and
                np.allclose(np.asarray(b_), 0.0)):
            nontrivial.add(nm)
    return nontrivial


def make_in_maps(inputs, nontrivial, n_cores=8, n_elems=2):
    Xq = np.asarray(inputs["query_modal"], np.float32)
    Xk = np.asarray(inputs["key_modal"], np.float32)
    Wq = np.asarray(inputs["Wq"], np.float32)
    Wk = np.asarray(inputs["Wk"], np.float32)
    Wv = np.asarray(inputs["Wv"], np.float32)
    Wo = np.asarray(inputs["Wo"], np.float32)
    wqk8 = _f8((Wq @ Wk.T) * SQ)
    wvo8 = _f8((Wv @ Wo) * SV)
    w1b = _bf16(inputs["W1"])
    w2b = _bf16(inputs["W2"])
    weights = {}
    if "bqk" in nontrivial:
        # folded: scores row-bias = bq @ Wk^T (the bk term is softmax-inv)
        weights["bqk"] = np.asarray(
            np.asarray(inputs["bq"], np.float32) @ Wk.T * SQ, np.float32)
    if "bv" in nontrivial:
        weights["bv"] = _bf16(
            (np.asarray(inputs["bv"], np.float32) @ Wo) * SV)
    if "bo" in nontrivial:
        weights["bo"] = np.asarray(inputs["bo"], np.float32)
    for nm in ("bf1", "bf2"):
        if nm in nontrivial:
            weights[nm] = _bf16(np.float32(inputs[nm]))
    for nm, (gk, bk_) in (("g1b1", ("g1", "b1")), ("gfbf", ("gf", "bf")),
                          ("g2b2", ("g2", "b2"))):
        if nm in nontrivial:
            weights[nm + "_g"] = np.asarray(inputs[gk], np.float32)
            weights[nm + "_b"] = np.asarray(inputs[bk_], np.float32)

    in_maps = []
    for c in range(n_cores):
        m = dict(weights)
        parts = []
        for e in range(n_elems):
            b = e * n_cores + c
            parts.append(_f8(np.ascontiguousarray(Xq[b].T)).ravel().view(np.uint8))
            parts.append(_f8(np.ascontiguousarray(Xk[b].T)).ravel().view(np.uint8))
        parts += [wqk8.ravel().view(np.uint8), wvo8.ravel().view(np.uint8)]
        for e in range(n_elems):
            b = e * n_cores + c
            parts.append(_bf16(Xq[b]).ravel().view(np.uint8))
        parts += [w1b.ravel().view(np.uint8), w2b.ravel().view(np.uint8)]
        m["in_u8"] = np.concatenate(parts)
        in_maps.append(m)
    return in_maps


def kernel(query_modal, key_modal, Wq, bq, Wk, bk, Wv, bv, Wo, bo,
           g1, b1, W1, bf1, gf, bf, W2, bf2, g2, b2):
    query_modal = np.asarray(query_modal, np.float32)
    B, N, D = query_modal.shape

    nontrivial = detect_nontrivial(bq, bk, bv, bo, bf1, bf2,
                                   g1, b1, gf, bf, g2, b2)
    n_cores = 8
    n_elems = (B + n_cores - 1) // n_cores
    runner = _get_runner(nontrivial, n_elems)

    inputs = dict(query_modal=query_modal, key_modal=key_modal,
                  Wq=Wq, bq=bq, Wk=Wk, bk=bk, Wv=Wv, bv=bv, Wo=Wo, bo=bo,
                  g1=g1, b1=b1, W1=W1, bf1=bf1, gf=gf, bf=bf,
                  W2=W2, bf2=bf2, g2=g2, b2=b2)
    in_maps = make_in_maps(inputs, nontrivial, n_cores, n_elems)
    bufs = runner.put_inputs(in_maps)
    outs = runner.run(bufs)
    res = runner.results(outs)
    y = np.empty((B, N, D), np.float32)
    for c in range(n_cores):
        yc = res[c]["y"].reshape(n_elems, N, D)
        for e in range(n_elems):
            y[e * n_cores + c] = yc[e]
    return np.ascontiguousarray(y)

